# revision 84
# baseline (speedup 1.0000x reference)
"""Trainium2 Bass kernel for AttentionFlowLayer (B=8, CS=1024, QS=128, D=1024).

Strategy: pure data-parallel over batch — core b computes batch b end to end,
no collectives.  Per core, the math is restructured to cut TensorEngine FLOPs:

  S[i,j] = ctx.(wcq*q + wc) |ij + q.w_q |j   (wc folded into the S operand;
  Pn     = softmax_j(S)                       alpha_b cancels and is dropped)
  qcw    = softmax_i(max_j S);  ch = qcw.ctx
  out    = Pn @ (q @ b2T + 1 x bias)          (rank-QS factorization; bias row
         + (ctx . query_hat) @ b3T             folded in via sum_j Pn[i,j]==1)
         + (ctx . ch) @ b4T                   [fp8e4m3 DoubleRow: 2 rows/cycle]
  bias   = ch @ b1T + beta_b

The b4 block runs in fp8 DoubleRow (x4 = 16*ctx.ch quantized on device, w4 =
512*b4T quantized on host) — half the PE cycles of bf16.  All other matmuls
stay bf16; their host-side weight streams are pre-scaled by K = 8192 so every
term accumulates in one PSUM group, descaled once in the PSUM->SBUF copy.
Output is written bf16 and upcast to f32 on host.
"""

import sys

sys.path.insert(0, "/opt/trn_rl_repo")

import numpy as np
import ml_dtypes

import concourse.bacc as bacc
import concourse.bass as bass
import concourse.mybir as mybir
import concourse.tile as tile
from concourse.bass_utils import run_bass_kernel_spmd

BF16 = mybir.dt.bfloat16
F32 = mybir.dt.float32
FP8 = mybir.dt.float8e4
NPBF16 = ml_dtypes.bfloat16
NPF8 = ml_dtypes.float8_e4m3
DR = mybir.MatmulPerfMode.DoubleRow

B, CS, QS, D = 8, 1024, 128, 1024
H8 = 4 * D
NC = D // 128   # d-chunks
NT = CS // 128  # i-tiles
NH = H8 // 512  # o-chunks
NP = NC // 2    # DoubleRow c-pairs
ts = bass.ts

SX = 16.0       # x4 fp8 scale
SW = 512.0      # w4 fp8 scale
KSC = SX * SW   # global PSUM product scale (bf16 streams pre-scaled by this)

TRACE = False
_LAST_EXEC_NS = None
_NC_CACHE = None


def _build():
    nc = bacc.Bacc("TRN2", target_bir_lowering=False, debug=False)

    # [p, c, i]: contextT -> SBUF [d128, c, i] (single tile, i = t*128+ii)
    d_ct = nc.dram_tensor("ct8", [128, NC, CS], BF16, kind="ExternalInput")
    # [p, t, d]: context natural, i on partitions
    d_cx = nc.dram_tensor("cx8", [128, NT, D], BF16, kind="ExternalInput")
    d_q = nc.dram_tensor("q", [QS, D], BF16, kind="ExternalInput")
    # [p, c, j]: queryT, d on partitions
    d_qT = nc.dram_tensor("qT8", [128, NC, QS], BF16, kind="ExternalInput")
    # host-precomputed S operands: qscp = wcq*qT + wc, qwb = q @ wq
    d_qscp = nc.dram_tensor("qscp8", [128, NC, QS], BF16, kind="ExternalInput")
    d_qwb = nc.dram_tensor("qwb8", [1, QS], BF16, kind="ExternalInput")
    d_id = nc.dram_tensor("identf", [128, 128], F32, kind="ExternalInput")
    # beta blocks pre-packed per output-chunk h, per-partition contiguous
    # [h, p, c, f]; bf16 streams pre-scaled by KSC (b1 by KSC/64), b4 fp8 by SW
    # b1 runs fp8 DoubleRow with a 64-col padded ch stationary (the [1,512]
    # out shape fails dual-fp8 LdWeights ISA checks; [64,512] passes)
    d_b1 = nc.dram_tensor("b1q", [NH, 128, NC, 512], FP8, kind="ExternalInput")
    d_b2 = nc.dram_tensor("b2h", [NH, 128, NC, 512], BF16, kind="ExternalInput")
    # b3 stays bf16: pushing more of the group into fp8 double-pump raises
    # average PE power past the DVFS threshold and drops the clock 2.4->2.0
    d_b3 = nc.dram_tensor("b3h", [NH, 128, NC, 512], BF16, kind="ExternalInput")
    d_b4 = nc.dram_tensor("b4h", [NH, 128, NC, 512], FP8, kind="ExternalInput")
    d_bb = nc.dram_tensor("bb", [1, H8], BF16, kind="ExternalInput")  # K*beta_b
    d_out = nc.dram_tensor("out", [CS, H8], BF16, kind="ExternalOutput")

    with tile.TileContext(nc) as tc:
        with tc.tile_pool(name="persist", bufs=1) as pp:
            # ---- persistent SBUF tensors -------------------------------
            CTall = pp.tile([128, NC, CS], BF16)    # contextT [d128, c, i]
            CX = pp.tile([128, NT, D], BF16)        # context natural [i128, t, d]
            B3T = pp.tile([128, NC, CS], BF16)      # (ctx * query_hat)^T [d, c, i]
            X4 = pp.tile([128, NC, CS], FP8)        # fp8 of 16*(ctx*ch)^T
            PnT = pp.tile([128, CS], BF16)          # softmax_j(S)^T  [j, i]
            Q = pp.tile([QS, D], BF16)              # query natural   [j, d]
            QT = pp.tile([128, NC, QS], BF16)       # queryT          [d, j]
            QSCp = pp.tile([128, NC, QS], BF16)     # queryT*wcq + wc (host)
            IDENT = pp.tile([128, 128], F32)
            BBr = pp.tile([1, NH, 512], BF16)
            QWB = pp.tile([1, QS], BF16)            # q . w_q as a row
            ONESb = pp.tile([1, 128], BF16)
            ONESC = pp.tile([128, 1], F32)
            ONESR = pp.tile([1, 128], F32)
            NEGMX = pp.tile([128, NT], F32)         # -max_j S, per i-tile col
            SM = pp.tile([128, NT], F32)
            RSM = pp.tile([128, NT], F32)
            ECOL = pp.tile([128, NT], F32)          # exp(mx)
            TOT = pp.tile([1, 1], F32)
            RTOT = pp.tile([1, 1], F32)
            RTOTB = pp.tile([128, 1], F32)
            QCWC = pp.tile([128, NT], BF16)         # qcw columns
            CH16 = pp.tile([128, NC], F32)          # 16*ch per-partition scalars
            CH64 = pp.tile([128, NC], F32)
            CHQP = pp.tile([128, NC, 64], FP8)      # fp8 64*ch bcast to 64 cols
            ONESF = pp.tile([128, 64], BF16)

            # ---- beta stream pool: fetches for h=0/1 issue first so the
            # gpsimd/scalar DMA queues start streaming immediately ------
            bwp_cm = tc.tile_pool(name="bwp", bufs=2)
            bwp = bwp_cm.__enter__()

            def fetch_beta(h):
                T3 = bwp.tile([128, NC, 512], BF16, tag="b3", name="T3")
                nc.gpsimd.dma_start(T3[:], d_b3[h])
                T4 = bwp.tile([128, NC, 512], FP8, tag="b4", name="T4")
                nc.gpsimd.dma_start(T4[:], d_b4[h])
                T2 = bwp.tile([128, NC, 512], BF16, tag="b2", name="T2")
                nc.scalar.dma_start(T2[:], d_b2[h])
                T1 = bwp.tile([128, NC, 512], FP8, tag="b1", name="T1")
                nc.scalar.dma_start(T1[:], d_b1[h])
                return (T3, T4, T2, T1)

            # ---- loads: input tensors get the HBM to themselves first;
            # the beta prefetch queues are gated on CTall's arrival via
            # tiny dummy reads so they don't steal startup bandwidth ----
            nc.sync.dma_start(QT[:], d_qT[:])
            nc.gpsimd.dma_start(QSCp[:], d_qscp[:])
            nc.gpsimd.dma_start(QWB[:], d_qwb[:])
            nc.gpsimd.dma_start(IDENT[:], d_id[:])
            nc.gpsimd.dma_start(BBr[:], d_bb[:].rearrange("o (h f) -> o h f", f=512))
            nc.sync.dma_start(CTall[:, 0:3, :], d_ct[:, 0:3, :])
            nc.scalar.dma_start(CTall[:, 3:6, :], d_ct[:, 3:6, :])
            nc.gpsimd.dma_start(CTall[:, 6:8, :], d_ct[:, 6:8, :])
            nc.sync.dma_start(Q[:], d_q[:])
            nc.sync.dma_start(CX[:], d_cx[:])

            GATE1 = pp.tile([128, 8], BF16, name="GATE1")
            GATE2 = pp.tile([128, 8], BF16, name="GATE2")
            nc.gpsimd.tensor_copy(GATE1[:], CTall[:, 0, 0:8])
            nc.scalar.copy(GATE2[:], CTall[:, 3, 0:8])

            beta_cur = fetch_beta(0)
            beta_next = fetch_beta(1)
            nc.vector.memset(ONESb[:], 1.0)
            nc.vector.memset(ONESC[:], 1.0)
            nc.vector.memset(ONESR[:], 1.0)
            nc.vector.memset(ONESF[:], 1.0)

            # ---- per-tile: S, softmax_j, PnT --------------------------
            with (
                tc.tile_pool(name="ps", bufs=4, space="PSUM") as ps,
                tc.tile_pool(name="pt", bufs=2, space="PSUM") as pt,
                tc.tile_pool(name="sp", bufs=8) as sp,
            ):
                # phase 1: S matmuls + row max + exp per tile (scalar does
                # only exps here, so nothing queues behind cross-engine work)
                P_sbs = []
                for t in range(NT):
                    PS_S = ps.tile([128, QS], F32, tag="s")
                    for c in range(NC):
                        nc.tensor.matmul(
                            PS_S[:], CTall[:, c, ts(t, 128)], QSCp[:, c, :],
                            start=(c == 0), stop=False,
                        )
                    nc.tensor.matmul(PS_S[:], ONESb[:], QWB[:], start=False, stop=True)
                    nc.vector.tensor_reduce(
                        NEGMX[:, t : t + 1], PS_S[:],
                        axis=mybir.AxisListType.X, op=mybir.AluOpType.max, negate=True,
                    )
                    P_sb = sp.tile([128, QS], F32, tag="p")
                    nc.scalar.activation(
                        P_sb[:], PS_S[:], mybir.ActivationFunctionType.Exp,
                        bias=NEGMX[:, t : t + 1], accum_out=SM[:, t : t + 1],
                    )
                    P_sbs.append(P_sb)
                # phase 2: normalize + transpose; PnT cast lands on scalar
                # (all exps already issued, so no serialization)
                for t in range(NT):
                    nc.vector.reciprocal(RSM[:, t : t + 1], SM[:, t : t + 1])
                    Pn_sb = sp.tile([128, QS], F32, tag="pn")
                    nc.vector.tensor_scalar_mul(
                        Pn_sb[:], P_sbs[t][:], RSM[:, t : t + 1]
                    )
                    PS_T = pt.tile([128, 128], F32, tag="t")
                    nc.tensor.transpose(PS_T[:], Pn_sb[:], IDENT[:])
                    nc.scalar.copy(PnT[:, ts(t, 128)], PS_T[:])

            # ---- softmax_i(max_j S) -> qcw, ch; qh; B3T ---------------
            nc.scalar.activation(
                ECOL[:], NEGMX[:], mybir.ActivationFunctionType.Exp, scale=-1.0
            )
            with (
                tc.tile_pool(name="pd", bufs=1, space="PSUM") as pd,
                tc.tile_pool(name="pg", bufs=3, space="PSUM") as pg,
                tc.tile_pool(name="pe", bufs=1, space="PSUM") as pe,
                tc.tile_pool(name="sp3", bufs=3) as sp3,
            ):
                PS_tot = pd.tile([1, NT], F32)
                nc.tensor.matmul(PS_tot[:], ONESC[:], ECOL[:])
                nc.vector.tensor_reduce(
                    TOT[:], PS_tot[:], axis=mybir.AxisListType.X, op=mybir.AluOpType.add
                )
                nc.vector.reciprocal(RTOT[:], TOT[:])
                PS_rb = pd.tile([128, 1], F32)
                nc.tensor.matmul(PS_rb[:], ONESR[:], RTOT[:])
                nc.vector.tensor_copy(RTOTB[:], PS_rb[:])
                nc.vector.tensor_scalar_mul(QCWC[:], ECOL[:], RTOTB[:])

                def qh_half(hh):
                    sl = slice(hh * 512, (hh + 1) * 512)
                    for c in range(NC):
                        PS_qh = pg.tile([128, 512], F32, tag="qh")
                        nc.tensor.matmul(PS_qh[:], Q[:, ts(c, 128)], PnT[:, sl])
                        QHB = sp3.tile([128, 512], BF16, tag="qhb", name="QHB")
                        nc.scalar.copy(QHB[:], PS_qh[:])
                        nc.vector.tensor_tensor(
                            B3T[:, c, sl], CTall[:, c, sl], QHB[:],
                            op=mybir.AluOpType.mult,
                        )

                # qh(hh=0) fills the PE while DVE finishes softmax; the ch
                # matmul chain then overlaps DVE's B3T(hh=0) work
                qh_half(0)
                PS_ch = pe.tile([128, NC], F32)
                for t in range(NT):
                    for c in range(NC):
                        nc.tensor.matmul(
                            PS_ch[:, c : c + 1], CX[:, t, ts(c, 128)], QCWC[:, t : t + 1],
                            start=(t == 0 and c == 0), stop=(t == NT - 1 and c == NC - 1),
                            skip_group_check=True,
                        )
                nc.vector.tensor_scalar_mul(CH16[:], PS_ch[:], SX)
                nc.vector.tensor_scalar_mul(CH64[:], PS_ch[:], 64.0)
                for c in range(NC):
                    nc.vector.tensor_scalar_mul(
                        CHQP[:, c, :], ONESF[:], CH64[:, c : c + 1]
                    )
                qh_half(1)
                # X4 = fp8(16 * ctxT * ch[d]) — batched [128, CS] ops split
                # across the vector and scalar engines
                for c in range(NC):
                    if c % 2 == 0:
                        nc.vector.tensor_scalar_mul(
                            X4[:, c, :], CTall[:, c, :], CH16[:, c : c + 1]
                        )
                    else:
                        nc.scalar.activation(
                            X4[:, c, :], CTall[:, c, :],
                            mybir.ActivationFunctionType.Copy,
                            scale=CH16[:, c : c + 1],
                        )

            # ---- fused output loop ------------------------------------
            with (
                tc.tile_pool(name="pw", bufs=1, space="PSUM") as pw,
                tc.tile_pool(name="pi", bufs=1, space="PSUM") as pi,
                tc.tile_pool(name="pj", bufs=6, space="PSUM") as pj,
                tc.tile_pool(name="whp", bufs=2) as whp,
                tc.tile_pool(name="op", bufs=4) as outp,
            ):
                def produce_w2h(h, T2, T1):
                    # biasK = 64ch @ 128b1 (fp8 DR, 64-col padded stationary;
                    # row 0 is the result) + K*beta_b folded into the copy
                    PS_b = pi.tile([64, 512], F32, tag="bi", name="PS_b")
                    for p in range(NP):
                        nc.tensor.matmul(
                            PS_b[:], CHQP[:, 2 * p : 2 * p + 2, :],
                            T1[:, 2 * p : 2 * p + 2, :],
                            start=(p == 0), stop=(p == NP - 1), perf_mode=DR,
                        )
                    BIH = whp.tile([1, 512], BF16, tag="bih", name="BIH")
                    nc.vector.tensor_tensor(
                        BIH[:], PS_b[0:1, :], BBr[:, h, :], op=mybir.AluOpType.add
                    )
                    PS_w2 = pw.tile([128, 512], F32, tag="w2", name="PS_w2")
                    for c in range(NC):
                        nc.tensor.matmul(
                            PS_w2[:], QT[:, c, :], T2[:, c, :],
                            start=(c == 0), stop=False,
                        )
                    nc.tensor.matmul(
                        PS_w2[:], ONESb[:], BIH[:], start=False, stop=True
                    )
                    W2H = whp.tile([128, 512], BF16, tag="w2h", name="W2H")
                    nc.vector.tensor_copy(W2H[:], PS_w2[:])
                    return W2H

                w2h_cur = produce_w2h(0, beta_cur[2], beta_cur[3])
                for h in range(NH):
                    T3, T4 = beta_cur[0], beta_cur[1]
                    w2h_nxt = None
                    for t in range(NT):
                        # mid-h: prefetch h+2 and produce W2H(h+1) so the
                        # next h's first stop-matmul never waits on them
                        if t == NT // 2 and h + 1 < NH:
                            if h + 2 < NH:
                                beta_next_new = fetch_beta(h + 2)
                            w2h_nxt = produce_w2h(h + 1, beta_next[2], beta_next[3])
                        PS_o = pj.tile([128, 512], F32, tag="o", name="PS_o")
                        for c in range(NC):
                            nc.tensor.matmul(
                                PS_o[:], B3T[:, c, ts(t, 128)], T3[:, c, :],
                                start=(c == 0), stop=False,
                            )
                        for p in range(NP):
                            nc.tensor.matmul(
                                PS_o[:], X4[:, 2 * p : 2 * p + 2, ts(t, 128)],
                                T4[:, 2 * p : 2 * p + 2, :],
                                start=False, stop=False, perf_mode=DR,
                            )
                        nc.tensor.matmul(
                            PS_o[:], PnT[:, ts(t, 128)], w2h_cur[:],
                            start=False, stop=True,
                        )
                        OS = outp.tile([128, 512], BF16, tag="os", name="OS")
                        if t % 2 == 0:
                            nc.vector.tensor_scalar_mul(OS[:], PS_o[:], 1.0 / KSC)
                        else:
                            nc.scalar.activation(
                                OS[:], PS_o[:],
                                mybir.ActivationFunctionType.Copy, scale=1.0 / KSC,
                            )
                        if t % 2 == 0:
                            nc.sync.dma_start(d_out[ts(t, 128), ts(h, 512)], OS[:])
                        else:
                            nc.scalar.dma_start(d_out[ts(t, 128), ts(h, 512)], OS[:])
                    if h + 1 < NH:
                        beta_cur = beta_next
                        if h + 2 < NH:
                            beta_next = beta_next_new
                        w2h_cur = w2h_nxt

            bwp_cm.__exit__(None, None, None)

    nc.compile()
    return nc


def _get_nc():
    global _NC_CACHE
    if _NC_CACHE is None:
        _NC_CACHE = _build()
    return _NC_CACHE


def _pack_beta(w):
    # [nch*128, 4096] (d, o) -> [NH, 128, nch, 512] per-partition contiguous
    nch = w.shape[0] // 128
    return np.ascontiguousarray(w.reshape(nch, 128, NH, 512).transpose(2, 1, 0, 3))


def _prep_shared(alpha_w, beta_w, beta_b):
    shared = {
        "identf": np.eye(128, dtype=np.float32),
        "bb": (KSC * beta_b).reshape(1, H8).astype(NPBF16),
    }
    betaT = np.ascontiguousarray(beta_w.T)  # [f, o] = [d, o]
    shared["b1q"] = _pack_beta(((KSC / 64.0) * betaT[0:D]).astype(NPF8))
    shared["b2h"] = _pack_beta((KSC * betaT[D : 2 * D]).astype(NPBF16))
    shared["b3h"] = _pack_beta((KSC * betaT[2 * D : 3 * D]).astype(NPBF16))
    shared["b4h"] = _pack_beta((SW * betaT[3 * D :]).astype(NPF8))
    return shared


def kernel(context, query, alpha_w, alpha_b, beta_w, beta_b):
    global _LAST_EXEC_NS
    context = np.asarray(context, dtype=np.float32)
    query = np.asarray(query, dtype=np.float32)
    alpha_w = np.asarray(alpha_w, dtype=np.float32)
    beta_w = np.asarray(beta_w, dtype=np.float32)
    beta_b = np.asarray(beta_b, dtype=np.float32)

    shared = _prep_shared(alpha_w, beta_w, beta_b)

    wc, wq, wcq = alpha_w[:D], alpha_w[D : 2 * D], alpha_w[2 * D :]
    in_maps = []
    for b in range(B):
        cb = context[b]
        qb = query[b]
        qT = qb.T  # [d, j]
        qscp = wcq[:, None] * qT + wc[:, None]
        m = {
            "qscp8": np.ascontiguousarray(
                qscp.reshape(NC, 128, QS).transpose(1, 0, 2)
            ).astype(NPBF16),
            "qwb8": (qb @ wq).reshape(1, QS).astype(NPBF16),
            # [t, ii, c, p] -> [p, c, t*128+ii]
            "ct8": np.ascontiguousarray(
                cb.reshape(NT, 128, NC, 128).transpose(3, 2, 0, 1).reshape(128, NC, CS)
            ).astype(NPBF16),
            # [t, ii, d] -> [ii, t, d]
            "cx8": np.ascontiguousarray(
                cb.reshape(NT, 128, D).transpose(1, 0, 2)
            ).astype(NPBF16),
            "q": qb.astype(NPBF16),
            # qT [d, j]: [c, p, j] -> [p, c, j]
            "qT8": np.ascontiguousarray(
                qb.T.reshape(NC, 128, QS).transpose(1, 0, 2)
            ).astype(NPBF16),
        }
        m.update(shared)
        in_maps.append(m)

    nc = _get_nc()
    res = run_bass_kernel_spmd(nc, in_maps, list(range(B)), trace=TRACE)
    _LAST_EXEC_NS = res.exec_time_ns
    out = np.stack(
        [res.results[b]["out"].astype(np.float32) for b in range(B)], axis=0
    )
    return out


# revision 86
# speedup vs baseline: 1.0005x; 1.0005x over previous
"""Trainium2 Bass kernel for AttentionFlowLayer (B=8, CS=1024, QS=128, D=1024).

Strategy: pure data-parallel over batch — core b computes batch b end to end,
no collectives.  Per core, the math is restructured to cut TensorEngine FLOPs:

  S[i,j] = ctx.(wcq*q + wc) |ij + q.w_q |j   (wc folded into the S operand;
  Pn     = softmax_j(S)                       alpha_b cancels and is dropped)
  qcw    = softmax_i(max_j S);  ch = qcw.ctx
  out    = Pn @ (q @ b2T + 1 x bias)          (rank-QS factorization; bias row
         + (ctx . query_hat) @ b3T             folded in via sum_j Pn[i,j]==1)
         + (ctx . ch) @ b4T                   [fp8e4m3 DoubleRow: 2 rows/cycle]
  bias   = ch @ b1T + beta_b

The b4 block runs in fp8 DoubleRow (x4 = 16*ctx.ch quantized on device, w4 =
512*b4T quantized on host) — half the PE cycles of bf16.  All other matmuls
stay bf16; their host-side weight streams are pre-scaled by K = 8192 so every
term accumulates in one PSUM group, descaled once in the PSUM->SBUF copy.
Output is written bf16 and upcast to f32 on host.
"""

import sys

sys.path.insert(0, "/opt/trn_rl_repo")

import numpy as np
import ml_dtypes

import concourse.bacc as bacc
import concourse.bass as bass
import concourse.mybir as mybir
import concourse.tile as tile
from concourse.bass_utils import run_bass_kernel_spmd

BF16 = mybir.dt.bfloat16
F32 = mybir.dt.float32
FP8 = mybir.dt.float8e4
NPBF16 = ml_dtypes.bfloat16
NPF8 = ml_dtypes.float8_e4m3
DR = mybir.MatmulPerfMode.DoubleRow

B, CS, QS, D = 8, 1024, 128, 1024
H8 = 4 * D
NC = D // 128   # d-chunks
NT = CS // 128  # i-tiles
NH = H8 // 512  # o-chunks
NP = NC // 2    # DoubleRow c-pairs
ts = bass.ts

SX = 16.0       # x4 fp8 scale
SW = 512.0      # w4 fp8 scale
KSC = SX * SW   # global PSUM product scale (bf16 streams pre-scaled by this)

TRACE = False
_LAST_EXEC_NS = None
_NC_CACHE = None


def _build():
    nc = bacc.Bacc("TRN2", target_bir_lowering=False, debug=False)

    # [p, c, i]: contextT -> SBUF [d128, c, i] (single tile, i = t*128+ii)
    d_ct = nc.dram_tensor("ct8", [128, NC, CS], BF16, kind="ExternalInput")
    # [p, t, d]: context natural, i on partitions
    d_cx = nc.dram_tensor("cx8", [128, NT, D], BF16, kind="ExternalInput")
    d_q = nc.dram_tensor("q", [QS, D], BF16, kind="ExternalInput")
    # [p, c, j]: queryT, d on partitions
    d_qT = nc.dram_tensor("qT8", [128, NC, QS], BF16, kind="ExternalInput")
    # host-precomputed S operands: qscp = wcq*qT + wc, qwb = q @ wq
    d_qscp = nc.dram_tensor("qscp8", [128, NC, QS], BF16, kind="ExternalInput")
    d_qwb = nc.dram_tensor("qwb8", [1, QS], BF16, kind="ExternalInput")
    d_id = nc.dram_tensor("identf", [128, 128], F32, kind="ExternalInput")
    # beta blocks pre-packed per output-chunk h, per-partition contiguous
    # [h, p, c, f]; bf16 streams pre-scaled by KSC (b1 by KSC/64), b4 fp8 by SW
    # b1 runs fp8 DoubleRow with a 64-col padded ch stationary (the [1,512]
    # out shape fails dual-fp8 LdWeights ISA checks; [64,512] passes)
    d_b1 = nc.dram_tensor("b1q", [NH, 128, NC, 512], FP8, kind="ExternalInput")
    d_b2 = nc.dram_tensor("b2h", [NH, 128, NC, 512], BF16, kind="ExternalInput")
    # b3 stays bf16: pushing more of the group into fp8 double-pump raises
    # average PE power past the DVFS threshold and drops the clock 2.4->2.0
    d_b3 = nc.dram_tensor("b3h", [NH, 128, NC, 512], BF16, kind="ExternalInput")
    d_b4 = nc.dram_tensor("b4h", [NH, 128, NC, 512], FP8, kind="ExternalInput")
    d_bb = nc.dram_tensor("bb", [1, H8], BF16, kind="ExternalInput")  # K*beta_b
    d_out = nc.dram_tensor("out", [CS, H8], BF16, kind="ExternalOutput")

    with tile.TileContext(nc) as tc:
        with tc.tile_pool(name="persist", bufs=1) as pp:
            # ---- persistent SBUF tensors -------------------------------
            CTall = pp.tile([128, NC, CS], BF16)    # contextT [d128, c, i]
            CX = pp.tile([128, NT, D], BF16)        # context natural [i128, t, d]
            B3T = pp.tile([128, NC, CS], BF16)      # (ctx * query_hat)^T [d, c, i]
            X4 = pp.tile([128, NC, CS], FP8)        # fp8 of 16*(ctx*ch)^T
            PnT = pp.tile([128, CS], BF16)          # softmax_j(S)^T  [j, i]
            Q = pp.tile([QS, D], BF16)              # query natural   [j, d]
            QT = pp.tile([128, NC, QS], BF16)       # queryT          [d, j]
            QSCp = pp.tile([128, NC, QS], BF16)     # queryT*wcq + wc (host)
            IDENT = pp.tile([128, 128], F32)
            BBr = pp.tile([1, NH, 512], BF16)
            QWB = pp.tile([1, QS], BF16)            # q . w_q as a row
            ONESb = pp.tile([1, 128], BF16)
            ONESC = pp.tile([128, 1], F32)
            ONESR = pp.tile([1, 128], F32)
            NEGMX = pp.tile([128, NT], F32)         # -max_j S, per i-tile col
            SM = pp.tile([128, NT], F32)
            RSM = pp.tile([128, NT], F32)
            ECOL = pp.tile([128, NT], F32)          # exp(mx)
            TOT = pp.tile([1, 1], F32)
            RTOT = pp.tile([1, 1], F32)
            RTOTB = pp.tile([128, 1], F32)
            QCWC = pp.tile([128, NT], BF16)         # qcw columns
            CH16 = pp.tile([128, NC], F32)          # 16*ch per-partition scalars
            CH64 = pp.tile([128, NC], F32)
            CHQP = pp.tile([128, NC, 64], FP8)      # fp8 64*ch bcast to 64 cols
            ONESF = pp.tile([128, 64], BF16)

            # ---- beta stream pool: fetches for h=0/1 issue first so the
            # gpsimd/scalar DMA queues start streaming immediately ------
            bwp_cm = tc.tile_pool(name="bwp", bufs=2)
            bwp = bwp_cm.__enter__()

            def fetch_beta(h):
                T3 = bwp.tile([128, NC, 512], BF16, tag="b3", name="T3")
                nc.gpsimd.dma_start(T3[:], d_b3[h])
                T4 = bwp.tile([128, NC, 512], FP8, tag="b4", name="T4")
                nc.gpsimd.dma_start(T4[:], d_b4[h])
                T2 = bwp.tile([128, NC, 512], BF16, tag="b2", name="T2")
                nc.scalar.dma_start(T2[:], d_b2[h])
                T1 = bwp.tile([128, NC, 512], FP8, tag="b1", name="T1")
                nc.scalar.dma_start(T1[:], d_b1[h])
                return (T3, T4, T2, T1)

            # ---- loads: input tensors get the HBM to themselves first;
            # the beta prefetch queues are gated on CTall's arrival via
            # tiny dummy reads so they don't steal startup bandwidth ----
            nc.sync.dma_start(QT[:], d_qT[:])
            nc.gpsimd.dma_start(QSCp[:], d_qscp[:])
            nc.gpsimd.dma_start(QWB[:], d_qwb[:])
            nc.gpsimd.dma_start(IDENT[:], d_id[:])
            nc.gpsimd.dma_start(BBr[:], d_bb[:].rearrange("o (h f) -> o h f", f=512))
            nc.sync.dma_start(CTall[:, 0:3, :], d_ct[:, 0:3, :])
            nc.scalar.dma_start(CTall[:, 3:6, :], d_ct[:, 3:6, :])
            nc.gpsimd.dma_start(CTall[:, 6:8, :], d_ct[:, 6:8, :])
            nc.sync.dma_start(Q[:], d_q[:])
            nc.sync.dma_start(CX[:], d_cx[:])

            GATE1 = pp.tile([128, 8], BF16, name="GATE1")
            GATE2 = pp.tile([128, 8], BF16, name="GATE2")
            nc.gpsimd.tensor_copy(GATE1[:], CTall[:, 0, 0:8])
            nc.scalar.copy(GATE2[:], CTall[:, 3, 0:8])

            beta_cur = fetch_beta(0)
            beta_next = fetch_beta(1)
            nc.vector.memset(ONESb[:], 1.0)
            nc.vector.memset(ONESC[:], 1.0)
            nc.vector.memset(ONESR[:], 1.0)
            nc.vector.memset(ONESF[:], 1.0)

            # ---- per-tile: S, softmax_j, PnT --------------------------
            with (
                tc.tile_pool(name="ps", bufs=4, space="PSUM") as ps,
                tc.tile_pool(name="pt", bufs=2, space="PSUM") as pt,
                tc.tile_pool(name="sp", bufs=8) as sp,
            ):
                # phase 1: S matmuls + row max + exp per tile (scalar does
                # only exps here, so nothing queues behind cross-engine work)
                P_sbs = []
                for t in range(NT):
                    PS_S = ps.tile([128, QS], F32, tag="s")
                    for c in range(NC):
                        nc.tensor.matmul(
                            PS_S[:], CTall[:, c, ts(t, 128)], QSCp[:, c, :],
                            start=(c == 0), stop=False,
                        )
                    nc.tensor.matmul(PS_S[:], ONESb[:], QWB[:], start=False, stop=True)
                    nc.vector.tensor_reduce(
                        NEGMX[:, t : t + 1], PS_S[:],
                        axis=mybir.AxisListType.X, op=mybir.AluOpType.max, negate=True,
                    )
                    P_sb = sp.tile([128, QS], F32, tag="p")
                    nc.scalar.activation(
                        P_sb[:], PS_S[:], mybir.ActivationFunctionType.Exp,
                        bias=NEGMX[:, t : t + 1], accum_out=SM[:, t : t + 1],
                    )
                    P_sbs.append(P_sb)
                # phase 2: normalize + transpose; PnT cast lands on scalar
                # (all exps already issued, so no serialization)
                for t in range(NT):
                    nc.vector.reciprocal(RSM[:, t : t + 1], SM[:, t : t + 1])
                    Pn_sb = sp.tile([128, QS], F32, tag="pn")
                    nc.vector.tensor_scalar_mul(
                        Pn_sb[:], P_sbs[t][:], RSM[:, t : t + 1]
                    )
                    PS_T = pt.tile([128, 128], F32, tag="t")
                    nc.tensor.transpose(PS_T[:], Pn_sb[:], IDENT[:])
                    nc.scalar.copy(PnT[:, ts(t, 128)], PS_T[:])

            # ---- softmax_i(max_j S) -> qcw, ch; qh; B3T ---------------
            nc.scalar.activation(
                ECOL[:], NEGMX[:], mybir.ActivationFunctionType.Exp, scale=-1.0
            )
            with (
                tc.tile_pool(name="pd", bufs=1, space="PSUM") as pd,
                tc.tile_pool(name="pg", bufs=3, space="PSUM") as pg,
                tc.tile_pool(name="pe", bufs=1, space="PSUM") as pe,
            ):
                PS_tot = pd.tile([1, NT], F32)
                nc.tensor.matmul(PS_tot[:], ONESC[:], ECOL[:])
                nc.vector.tensor_reduce(
                    TOT[:], PS_tot[:], axis=mybir.AxisListType.X, op=mybir.AluOpType.add
                )
                nc.vector.reciprocal(RTOT[:], TOT[:])
                PS_rb = pd.tile([128, 1], F32)
                nc.tensor.matmul(PS_rb[:], ONESR[:], RTOT[:])
                nc.vector.tensor_copy(RTOTB[:], PS_rb[:])
                nc.vector.tensor_scalar_mul(QCWC[:], ECOL[:], RTOTB[:])

                def qh_half(hh):
                    sl = slice(hh * 512, (hh + 1) * 512)
                    for c in range(NC):
                        PS_qh = pg.tile([128, 512], F32, tag="qh")
                        nc.tensor.matmul(PS_qh[:], Q[:, ts(c, 128)], PnT[:, sl])
                        nc.vector.tensor_tensor(
                            B3T[:, c, sl], CTall[:, c, sl], PS_qh[:],
                            op=mybir.AluOpType.mult,
                        )

                # qh(hh=0) fills the PE while DVE finishes softmax; the ch
                # matmul chain then overlaps DVE's B3T(hh=0) work
                qh_half(0)
                PS_ch = pe.tile([128, NC], F32)
                for t in range(NT):
                    for c in range(NC):
                        nc.tensor.matmul(
                            PS_ch[:, c : c + 1], CX[:, t, ts(c, 128)], QCWC[:, t : t + 1],
                            start=(t == 0 and c == 0), stop=(t == NT - 1 and c == NC - 1),
                            skip_group_check=True,
                        )
                nc.vector.tensor_scalar_mul(CH16[:], PS_ch[:], SX)
                nc.vector.tensor_scalar_mul(CH64[:], PS_ch[:], 64.0)
                for c in range(NC):
                    nc.vector.tensor_scalar_mul(
                        CHQP[:, c, :], ONESF[:], CH64[:, c : c + 1]
                    )
                qh_half(1)
                # X4 = fp8(16 * ctxT * ch[d]) — batched [128, CS] ops split
                # across the vector and scalar engines
                for c in range(NC):
                    if c % 2 == 0:
                        nc.vector.tensor_scalar_mul(
                            X4[:, c, :], CTall[:, c, :], CH16[:, c : c + 1]
                        )
                    else:
                        nc.scalar.activation(
                            X4[:, c, :], CTall[:, c, :],
                            mybir.ActivationFunctionType.Copy,
                            scale=CH16[:, c : c + 1],
                        )

            # ---- fused output loop ------------------------------------
            with (
                tc.tile_pool(name="pw", bufs=1, space="PSUM") as pw,
                tc.tile_pool(name="pi", bufs=1, space="PSUM") as pi,
                tc.tile_pool(name="pj", bufs=6, space="PSUM") as pj,
                tc.tile_pool(name="whp", bufs=2) as whp,
                tc.tile_pool(name="op", bufs=4) as outp,
            ):
                def produce_w2h(h, T2, T1):
                    # biasK = 64ch @ 128b1 (fp8 DR, 64-col padded stationary;
                    # row 0 is the result) + K*beta_b folded into the copy
                    PS_b = pi.tile([64, 512], F32, tag="bi", name="PS_b")
                    for p in range(NP):
                        nc.tensor.matmul(
                            PS_b[:], CHQP[:, 2 * p : 2 * p + 2, :],
                            T1[:, 2 * p : 2 * p + 2, :],
                            start=(p == 0), stop=(p == NP - 1), perf_mode=DR,
                        )
                    BIH = whp.tile([1, 512], BF16, tag="bih", name="BIH")
                    nc.vector.tensor_tensor(
                        BIH[:], PS_b[0:1, :], BBr[:, h, :], op=mybir.AluOpType.add
                    )
                    PS_w2 = pw.tile([128, 512], F32, tag="w2", name="PS_w2")
                    for c in range(NC):
                        nc.tensor.matmul(
                            PS_w2[:], QT[:, c, :], T2[:, c, :],
                            start=(c == 0), stop=False,
                        )
                    nc.tensor.matmul(
                        PS_w2[:], ONESb[:], BIH[:], start=False, stop=True
                    )
                    W2H = whp.tile([128, 512], BF16, tag="w2h", name="W2H")
                    nc.vector.tensor_copy(W2H[:], PS_w2[:])
                    return W2H

                w2h_cur = produce_w2h(0, beta_cur[2], beta_cur[3])
                for h in range(NH):
                    T3, T4 = beta_cur[0], beta_cur[1]
                    w2h_nxt = None
                    for t in range(NT):
                        # mid-h: prefetch h+2 and produce W2H(h+1) so the
                        # next h's first stop-matmul never waits on them
                        if t == NT // 2 and h + 1 < NH:
                            if h + 2 < NH:
                                beta_next_new = fetch_beta(h + 2)
                            w2h_nxt = produce_w2h(h + 1, beta_next[2], beta_next[3])
                        PS_o = pj.tile([128, 512], F32, tag="o", name="PS_o")
                        for c in range(NC):
                            nc.tensor.matmul(
                                PS_o[:], B3T[:, c, ts(t, 128)], T3[:, c, :],
                                start=(c == 0), stop=False,
                            )
                        for p in range(NP):
                            nc.tensor.matmul(
                                PS_o[:], X4[:, 2 * p : 2 * p + 2, ts(t, 128)],
                                T4[:, 2 * p : 2 * p + 2, :],
                                start=False, stop=False, perf_mode=DR,
                            )
                        nc.tensor.matmul(
                            PS_o[:], PnT[:, ts(t, 128)], w2h_cur[:],
                            start=False, stop=True,
                        )
                        OS = outp.tile([128, 512], BF16, tag="os", name="OS")
                        if t % 2 == 0:
                            nc.vector.tensor_scalar_mul(OS[:], PS_o[:], 1.0 / KSC)
                        else:
                            nc.scalar.activation(
                                OS[:], PS_o[:],
                                mybir.ActivationFunctionType.Copy, scale=1.0 / KSC,
                            )
                        if t % 2 == 0:
                            nc.sync.dma_start(d_out[ts(t, 128), ts(h, 512)], OS[:])
                        else:
                            nc.scalar.dma_start(d_out[ts(t, 128), ts(h, 512)], OS[:])
                    if h + 1 < NH:
                        beta_cur = beta_next
                        if h + 2 < NH:
                            beta_next = beta_next_new
                        w2h_cur = w2h_nxt

            bwp_cm.__exit__(None, None, None)

    nc.compile()
    return nc


def _get_nc():
    global _NC_CACHE
    if _NC_CACHE is None:
        _NC_CACHE = _build()
    return _NC_CACHE


def _pack_beta(w):
    # [nch*128, 4096] (d, o) -> [NH, 128, nch, 512] per-partition contiguous
    nch = w.shape[0] // 128
    return np.ascontiguousarray(w.reshape(nch, 128, NH, 512).transpose(2, 1, 0, 3))


def _prep_shared(alpha_w, beta_w, beta_b):
    shared = {
        "identf": np.eye(128, dtype=np.float32),
        "bb": (KSC * beta_b).reshape(1, H8).astype(NPBF16),
    }
    betaT = np.ascontiguousarray(beta_w.T)  # [f, o] = [d, o]
    shared["b1q"] = _pack_beta(((KSC / 64.0) * betaT[0:D]).astype(NPF8))
    shared["b2h"] = _pack_beta((KSC * betaT[D : 2 * D]).astype(NPBF16))
    shared["b3h"] = _pack_beta((KSC * betaT[2 * D : 3 * D]).astype(NPBF16))
    shared["b4h"] = _pack_beta((SW * betaT[3 * D :]).astype(NPF8))
    return shared


def kernel(context, query, alpha_w, alpha_b, beta_w, beta_b):
    global _LAST_EXEC_NS
    context = np.asarray(context, dtype=np.float32)
    query = np.asarray(query, dtype=np.float32)
    alpha_w = np.asarray(alpha_w, dtype=np.float32)
    beta_w = np.asarray(beta_w, dtype=np.float32)
    beta_b = np.asarray(beta_b, dtype=np.float32)

    shared = _prep_shared(alpha_w, beta_w, beta_b)

    wc, wq, wcq = alpha_w[:D], alpha_w[D : 2 * D], alpha_w[2 * D :]
    in_maps = []
    for b in range(B):
        cb = context[b]
        qb = query[b]
        qT = qb.T  # [d, j]
        qscp = wcq[:, None] * qT + wc[:, None]
        m = {
            "qscp8": np.ascontiguousarray(
                qscp.reshape(NC, 128, QS).transpose(1, 0, 2)
            ).astype(NPBF16),
            "qwb8": (qb @ wq).reshape(1, QS).astype(NPBF16),
            # [t, ii, c, p] -> [p, c, t*128+ii]
            "ct8": np.ascontiguousarray(
                cb.reshape(NT, 128, NC, 128).transpose(3, 2, 0, 1).reshape(128, NC, CS)
            ).astype(NPBF16),
            # [t, ii, d] -> [ii, t, d]
            "cx8": np.ascontiguousarray(
                cb.reshape(NT, 128, D).transpose(1, 0, 2)
            ).astype(NPBF16),
            "q": qb.astype(NPBF16),
            # qT [d, j]: [c, p, j] -> [p, c, j]
            "qT8": np.ascontiguousarray(
                qb.T.reshape(NC, 128, QS).transpose(1, 0, 2)
            ).astype(NPBF16),
        }
        m.update(shared)
        in_maps.append(m)

    nc = _get_nc()
    res = run_bass_kernel_spmd(nc, in_maps, list(range(B)), trace=TRACE)
    _LAST_EXEC_NS = res.exec_time_ns
    out = np.stack(
        [res.results[b]["out"].astype(np.float32) for b in range(B)], axis=0
    )
    return out


# revision 89
# speedup vs baseline: 1.0043x; 1.0038x over previous
"""Trainium2 Bass kernel for AttentionFlowLayer (B=8, CS=1024, QS=128, D=1024).

Strategy: pure data-parallel over batch — core b computes batch b end to end,
no collectives.  Per core, the math is restructured to cut TensorEngine FLOPs:

  S[i,j] = ctx.(wcq*q + wc) |ij + q.w_q |j   (wc folded into the S operand;
  Pn     = softmax_j(S)                       alpha_b cancels and is dropped)
  qcw    = softmax_i(max_j S);  ch = qcw.ctx
  out    = Pn @ (q @ b2T + 1 x bias)          (rank-QS factorization; bias row
         + (ctx . query_hat) @ b3T             folded in via sum_j Pn[i,j]==1)
         + (ctx . ch) @ b4T                   [fp8e4m3 DoubleRow: 2 rows/cycle]
  bias   = ch @ b1T + beta_b

The b4 block runs in fp8 DoubleRow (x4 = 16*ctx.ch quantized on device, w4 =
512*b4T quantized on host) — half the PE cycles of bf16.  All other matmuls
stay bf16; their host-side weight streams are pre-scaled by K = 8192 so every
term accumulates in one PSUM group, descaled once in the PSUM->SBUF copy.
Output is written bf16 and upcast to f32 on host.
"""

import sys

sys.path.insert(0, "/opt/trn_rl_repo")

import numpy as np
import ml_dtypes

import concourse.bacc as bacc
import concourse.bass as bass
import concourse.mybir as mybir
import concourse.tile as tile
from concourse.bass_utils import run_bass_kernel_spmd

BF16 = mybir.dt.bfloat16
F32 = mybir.dt.float32
FP8 = mybir.dt.float8e4
NPBF16 = ml_dtypes.bfloat16
NPF8 = ml_dtypes.float8_e4m3
DR = mybir.MatmulPerfMode.DoubleRow

B, CS, QS, D = 8, 1024, 128, 1024
H8 = 4 * D
NC = D // 128   # d-chunks
NT = CS // 128  # i-tiles
NH = H8 // 512  # o-chunks
NP = NC // 2    # DoubleRow c-pairs
ts = bass.ts

SX = 16.0       # x4 fp8 scale
SW = 512.0      # w4 fp8 scale
KSC = SX * SW   # global PSUM product scale (bf16 streams pre-scaled by this)

TRACE = False
_LAST_EXEC_NS = None
_NC_CACHE = None


def _build():
    nc = bacc.Bacc("TRN2", target_bir_lowering=False, debug=False)

    # [p, c, i]: contextT -> SBUF [d128, c, i] (single tile, i = t*128+ii)
    d_ct = nc.dram_tensor("ct8", [128, NC, CS], BF16, kind="ExternalInput")
    # [p, t, d]: context natural, i on partitions
    d_cx = nc.dram_tensor("cx8", [128, NT, D], BF16, kind="ExternalInput")
    d_q = nc.dram_tensor("q", [QS, D], BF16, kind="ExternalInput")
    # [p, c, j]: queryT, d on partitions
    d_qT = nc.dram_tensor("qT8", [128, NC, QS], BF16, kind="ExternalInput")
    # host-precomputed S operands: qscp = wcq*qT + wc, qwb = q @ wq
    d_qscp = nc.dram_tensor("qscp8", [128, NC, QS], BF16, kind="ExternalInput")
    d_qwb = nc.dram_tensor("qwb8", [1, QS], BF16, kind="ExternalInput")
    d_id = nc.dram_tensor("identf", [128, 128], F32, kind="ExternalInput")
    # beta blocks pre-packed per output-chunk h, per-partition contiguous
    # [h, p, c, f]; bf16 streams pre-scaled by KSC (b1 by KSC/64), b4 fp8 by SW
    # b1 runs fp8 DoubleRow with a 64-col padded ch stationary (the [1,512]
    # out shape fails dual-fp8 LdWeights ISA checks; [64,512] passes)
    d_b1 = nc.dram_tensor("b1q", [NH, 128, NC, 512], FP8, kind="ExternalInput")
    d_b2 = nc.dram_tensor("b2h", [NH, 128, NC, 512], BF16, kind="ExternalInput")
    # b3 stays bf16: pushing more of the group into fp8 double-pump raises
    # average PE power past the DVFS threshold and drops the clock 2.4->2.0
    d_b3 = nc.dram_tensor("b3h", [NH, 128, NC, 512], BF16, kind="ExternalInput")
    d_b4 = nc.dram_tensor("b4h", [NH, 128, NC, 512], FP8, kind="ExternalInput")
    d_bb = nc.dram_tensor("bb", [1, H8], BF16, kind="ExternalInput")  # K*beta_b
    d_out = nc.dram_tensor("out", [CS, H8], BF16, kind="ExternalOutput")

    with tile.TileContext(nc) as tc:
        with tc.tile_pool(name="persist", bufs=1) as pp:
            # ---- persistent SBUF tensors -------------------------------
            CTall = pp.tile([128, NC, CS], BF16)    # contextT [d128, c, i]
            CX = pp.tile([128, NT, D], BF16)        # context natural [i128, t, d]
            B3T = pp.tile([128, NC, CS], BF16)      # (ctx * query_hat)^T [d, c, i]
            X4 = pp.tile([128, NC, CS], FP8)        # fp8 of 16*(ctx*ch)^T
            PnT = pp.tile([128, CS], BF16)          # softmax_j(S)^T  [j, i]
            Q = pp.tile([QS, D], BF16)              # query natural   [j, d]
            QT = pp.tile([128, NC, QS], BF16)       # queryT          [d, j]
            QSCp = pp.tile([128, NC, QS], BF16)     # queryT*wcq + wc (host)
            IDENT = pp.tile([128, 128], F32)
            BBr = pp.tile([1, NH, 512], BF16)
            QWB = pp.tile([1, QS], BF16)            # q . w_q as a row
            ONESb = pp.tile([1, 128], BF16)
            ONESC = pp.tile([128, 1], F32)
            ONESR = pp.tile([1, 128], F32)
            NEGMX = pp.tile([128, NT], F32)         # -max_j S, per i-tile col
            SM = pp.tile([128, NT], F32)
            RSM = pp.tile([128, NT], F32)
            ECOL = pp.tile([128, NT], F32)          # exp(mx)
            TOT = pp.tile([1, 1], F32)
            RTOT = pp.tile([1, 1], F32)
            RTOTB = pp.tile([128, 1], F32)
            QCWC = pp.tile([128, NT], BF16)         # qcw columns
            CH16 = pp.tile([128, NC], F32)          # 16*ch per-partition scalars
            CH64 = pp.tile([128, NC], F32)
            CHQP = pp.tile([128, NC, 64], FP8)      # fp8 64*ch bcast to 64 cols
            ONESF = pp.tile([128, 64], BF16)

            # ---- beta stream pool: fetches for h=0/1 issue first so the
            # gpsimd/scalar DMA queues start streaming immediately ------
            bwp_cm = tc.tile_pool(name="bwp", bufs=2)
            bwp = bwp_cm.__enter__()

            def fetch_beta(h):
                T3 = bwp.tile([128, NC, 512], BF16, tag="b3", name="T3")
                nc.gpsimd.dma_start(T3[:], d_b3[h])
                T4 = bwp.tile([128, NC, 512], FP8, tag="b4", name="T4")
                nc.gpsimd.dma_start(T4[:], d_b4[h])
                T2 = bwp.tile([128, NC, 512], BF16, tag="b2", name="T2")
                nc.scalar.dma_start(T2[:], d_b2[h])
                T1 = bwp.tile([128, NC, 512], FP8, tag="b1", name="T1")
                nc.scalar.dma_start(T1[:], d_b1[h])
                return (T3, T4, T2, T1)

            # ---- loads: input tensors get the HBM to themselves first;
            # the beta prefetch queues are gated on CTall's arrival via
            # tiny dummy reads so they don't steal startup bandwidth ----
            nc.sync.dma_start(QT[:], d_qT[:])
            nc.gpsimd.dma_start(QSCp[:], d_qscp[:])
            nc.gpsimd.dma_start(QWB[:], d_qwb[:])
            nc.gpsimd.dma_start(IDENT[:], d_id[:])
            nc.gpsimd.dma_start(BBr[:], d_bb[:].rearrange("o (h f) -> o h f", f=512))
            nc.sync.dma_start(CTall[:, 0:3, :], d_ct[:, 0:3, :])
            nc.scalar.dma_start(CTall[:, 3:6, :], d_ct[:, 3:6, :])
            nc.gpsimd.dma_start(CTall[:, 6:8, :], d_ct[:, 6:8, :])
            nc.sync.dma_start(Q[:], d_q[:])
            nc.sync.dma_start(CX[:], d_cx[:])

            GATE1 = pp.tile([128, 8], BF16, name="GATE1")
            GATE2 = pp.tile([128, 8], BF16, name="GATE2")
            nc.gpsimd.tensor_copy(GATE1[:], CTall[:, 0, 0:8])
            nc.scalar.copy(GATE2[:], CTall[:, 3, 0:8])

            beta_cur = fetch_beta(0)
            beta_next = fetch_beta(1)
            nc.vector.memset(ONESb[:], 1.0)
            nc.vector.memset(ONESC[:], 1.0)
            nc.vector.memset(ONESR[:], 1.0)
            nc.vector.memset(ONESF[:], 1.0)

            # ---- per-tile: S, softmax_j, PnT --------------------------
            with (
                tc.tile_pool(name="ps", bufs=4, space="PSUM") as ps,
                tc.tile_pool(name="pt", bufs=2, space="PSUM") as pt,
                tc.tile_pool(name="sp", bufs=8) as sp,
            ):
                # phase 1: S matmuls + row max + exp per tile (scalar does
                # only exps here, so nothing queues behind cross-engine work)
                P_sbs = []
                for t in range(NT):
                    PS_S = ps.tile([128, QS], F32, tag="s")
                    for c in range(NC):
                        nc.tensor.matmul(
                            PS_S[:], CTall[:, c, ts(t, 128)], QSCp[:, c, :],
                            start=(c == 0), stop=False,
                        )
                    nc.tensor.matmul(PS_S[:], ONESb[:], QWB[:], start=False, stop=True)
                    nc.vector.tensor_reduce(
                        NEGMX[:, t : t + 1], PS_S[:],
                        axis=mybir.AxisListType.X, op=mybir.AluOpType.max, negate=True,
                    )
                    P_sb = sp.tile([128, QS], F32, tag="p")
                    nc.scalar.activation(
                        P_sb[:], PS_S[:], mybir.ActivationFunctionType.Exp,
                        bias=NEGMX[:, t : t + 1], accum_out=SM[:, t : t + 1],
                    )
                    P_sbs.append(P_sb)
                # phase 2: normalize + transpose; PnT cast lands on scalar
                # (all exps already issued, so no serialization)
                for t in range(NT):
                    nc.vector.reciprocal(RSM[:, t : t + 1], SM[:, t : t + 1])
                    Pn_sb = sp.tile([128, QS], F32, tag="pn")
                    nc.vector.tensor_scalar_mul(
                        Pn_sb[:], P_sbs[t][:], RSM[:, t : t + 1]
                    )
                    PS_T = pt.tile([128, 128], F32, tag="t")
                    nc.tensor.transpose(PS_T[:], Pn_sb[:], IDENT[:])
                    nc.scalar.copy(PnT[:, ts(t, 128)], PS_T[:])

            # ---- softmax_i(max_j S) -> qcw, ch; qh; B3T ---------------
            nc.scalar.activation(
                ECOL[:], NEGMX[:], mybir.ActivationFunctionType.Exp, scale=-1.0
            )
            with (
                tc.tile_pool(name="pd", bufs=1, space="PSUM") as pd,
                tc.tile_pool(name="pg", bufs=3, space="PSUM") as pg,
                tc.tile_pool(name="pe", bufs=1, space="PSUM") as pe,
            ):
                PS_tot = pd.tile([1, NT], F32)
                nc.tensor.matmul(PS_tot[:], ONESC[:], ECOL[:])
                nc.vector.tensor_reduce(
                    TOT[:], PS_tot[:], axis=mybir.AxisListType.X, op=mybir.AluOpType.add
                )
                nc.vector.reciprocal(RTOT[:], TOT[:])
                PS_rb = pd.tile([128, 1], F32)
                nc.tensor.matmul(PS_rb[:], ONESR[:], RTOT[:])
                nc.vector.tensor_copy(RTOTB[:], PS_rb[:])
                nc.vector.tensor_scalar_mul(QCWC[:], ECOL[:], RTOTB[:])

                def qh_half(hh):
                    sl = slice(hh * 512, (hh + 1) * 512)
                    for c in range(NC):
                        PS_qh = pg.tile([128, 512], F32, tag="qh")
                        nc.tensor.matmul(PS_qh[:], Q[:, ts(c, 128)], PnT[:, sl])
                        nc.vector.tensor_tensor(
                            B3T[:, c, sl], CTall[:, c, sl], PS_qh[:],
                            op=mybir.AluOpType.mult,
                        )

                # qh(hh=0) fills the PE while DVE finishes softmax; the ch
                # matmul chain then overlaps DVE's B3T(hh=0) work
                qh_half(0)
                PS_ch = pe.tile([128, NC], F32)
                for t in range(NT):
                    for c in range(NC):
                        nc.tensor.matmul(
                            PS_ch[:, c : c + 1], CX[:, t, ts(c, 128)], QCWC[:, t : t + 1],
                            start=(t == 0 and c == 0), stop=(t == NT - 1 and c == NC - 1),
                            skip_group_check=True,
                        )
                nc.vector.tensor_scalar_mul(CH16[:], PS_ch[:], SX)
                nc.vector.tensor_scalar_mul(CH64[:], PS_ch[:], 64.0)
                for c in range(NC):
                    nc.vector.tensor_scalar_mul(
                        CHQP[:, c, :], ONESF[:], CH64[:, c : c + 1]
                    )
                qh_half(1)
                # X4 = fp8(16 * ctxT * ch[d]) — batched [128, CS] ops split
                # across the vector and scalar engines
                for c in range(NC):
                    if c % 2 == 0:
                        nc.vector.tensor_scalar_mul(
                            X4[:, c, :], CTall[:, c, :], CH16[:, c : c + 1]
                        )
                    else:
                        nc.scalar.activation(
                            X4[:, c, :], CTall[:, c, :],
                            mybir.ActivationFunctionType.Copy,
                            scale=CH16[:, c : c + 1],
                        )

            # ---- fused output loop ------------------------------------
            with (
                tc.tile_pool(name="pw", bufs=1, space="PSUM") as pw,
                tc.tile_pool(name="pj", bufs=7, space="PSUM") as pj,
                tc.tile_pool(name="whp", bufs=2) as whp,
                tc.tile_pool(name="op", bufs=4) as outp,
            ):
                def produce_w2h(h, T2, T1):
                    # biasK = 64ch @ 128b1 (fp8 DR, 64-col padded stationary;
                    # row 0 is the result) + K*beta_b folded into the copy
                    PS_b = pw.tile([128, 512], F32, tag="wb", name="PS_b")
                    for p in range(NP):
                        nc.tensor.matmul(
                            PS_b[0:64, :], CHQP[:, 2 * p : 2 * p + 2, :],
                            T1[:, 2 * p : 2 * p + 2, :],
                            start=(p == 0), stop=(p == NP - 1), perf_mode=DR,
                        )
                    BIH = whp.tile([1, 512], BF16, tag="bih", name="BIH")
                    nc.vector.tensor_tensor(
                        BIH[:], PS_b[0:1, :], BBr[:, h, :], op=mybir.AluOpType.add
                    )
                    PS_w2 = pw.tile([128, 512], F32, tag="wb", name="PS_w2")
                    for c in range(NC):
                        nc.tensor.matmul(
                            PS_w2[:], QT[:, c, :], T2[:, c, :],
                            start=(c == 0), stop=False,
                        )
                    nc.tensor.matmul(
                        PS_w2[:], ONESb[:], BIH[:], start=False, stop=True
                    )
                    W2H = whp.tile([128, 512], BF16, tag="w2h", name="W2H")
                    nc.vector.tensor_copy(W2H[:], PS_w2[:])
                    return W2H

                w2h_cur = produce_w2h(0, beta_cur[2], beta_cur[3])
                for h in range(NH):
                    T3, T4 = beta_cur[0], beta_cur[1]
                    w2h_nxt = None
                    for t in range(NT):
                        # mid-h: prefetch h+2 and produce W2H(h+1) so the
                        # next h's first stop-matmul never waits on them
                        if t == NT // 2 and h + 1 < NH:
                            if h + 2 < NH:
                                beta_next_new = fetch_beta(h + 2)
                            w2h_nxt = produce_w2h(h + 1, beta_next[2], beta_next[3])
                        PS_o = pj.tile([128, 512], F32, tag="o", name="PS_o")
                        for c in range(NC):
                            nc.tensor.matmul(
                                PS_o[:], B3T[:, c, ts(t, 128)], T3[:, c, :],
                                start=(c == 0), stop=False,
                            )
                        for p in range(NP):
                            nc.tensor.matmul(
                                PS_o[:], X4[:, 2 * p : 2 * p + 2, ts(t, 128)],
                                T4[:, 2 * p : 2 * p + 2, :],
                                start=False, stop=False, perf_mode=DR,
                            )
                        nc.tensor.matmul(
                            PS_o[:], PnT[:, ts(t, 128)], w2h_cur[:],
                            start=False, stop=True,
                        )
                        OS = outp.tile([128, 512], BF16, tag="os", name="OS")
                        if t % 2 == 0:
                            nc.vector.tensor_scalar_mul(OS[:], PS_o[:], 1.0 / KSC)
                        else:
                            nc.scalar.activation(
                                OS[:], PS_o[:],
                                mybir.ActivationFunctionType.Copy, scale=1.0 / KSC,
                            )
                        if t % 2 == 0:
                            nc.sync.dma_start(d_out[ts(t, 128), ts(h, 512)], OS[:])
                        else:
                            nc.scalar.dma_start(d_out[ts(t, 128), ts(h, 512)], OS[:])
                    if h + 1 < NH:
                        beta_cur = beta_next
                        if h + 2 < NH:
                            beta_next = beta_next_new
                        w2h_cur = w2h_nxt

            bwp_cm.__exit__(None, None, None)

    nc.compile()
    return nc


def _get_nc():
    global _NC_CACHE
    if _NC_CACHE is None:
        _NC_CACHE = _build()
    return _NC_CACHE


def _pack_beta(w):
    # [nch*128, 4096] (d, o) -> [NH, 128, nch, 512] per-partition contiguous
    nch = w.shape[0] // 128
    return np.ascontiguousarray(w.reshape(nch, 128, NH, 512).transpose(2, 1, 0, 3))


def _prep_shared(alpha_w, beta_w, beta_b):
    shared = {
        "identf": np.eye(128, dtype=np.float32),
        "bb": (KSC * beta_b).reshape(1, H8).astype(NPBF16),
    }
    betaT = np.ascontiguousarray(beta_w.T)  # [f, o] = [d, o]
    shared["b1q"] = _pack_beta(((KSC / 64.0) * betaT[0:D]).astype(NPF8))
    shared["b2h"] = _pack_beta((KSC * betaT[D : 2 * D]).astype(NPBF16))
    shared["b3h"] = _pack_beta((KSC * betaT[2 * D : 3 * D]).astype(NPBF16))
    shared["b4h"] = _pack_beta((SW * betaT[3 * D :]).astype(NPF8))
    return shared


def kernel(context, query, alpha_w, alpha_b, beta_w, beta_b):
    global _LAST_EXEC_NS
    context = np.asarray(context, dtype=np.float32)
    query = np.asarray(query, dtype=np.float32)
    alpha_w = np.asarray(alpha_w, dtype=np.float32)
    beta_w = np.asarray(beta_w, dtype=np.float32)
    beta_b = np.asarray(beta_b, dtype=np.float32)

    shared = _prep_shared(alpha_w, beta_w, beta_b)

    wc, wq, wcq = alpha_w[:D], alpha_w[D : 2 * D], alpha_w[2 * D :]
    in_maps = []
    for b in range(B):
        cb = context[b]
        qb = query[b]
        qT = qb.T  # [d, j]
        qscp = wcq[:, None] * qT + wc[:, None]
        m = {
            "qscp8": np.ascontiguousarray(
                qscp.reshape(NC, 128, QS).transpose(1, 0, 2)
            ).astype(NPBF16),
            "qwb8": (qb @ wq).reshape(1, QS).astype(NPBF16),
            # [t, ii, c, p] -> [p, c, t*128+ii]
            "ct8": np.ascontiguousarray(
                cb.reshape(NT, 128, NC, 128).transpose(3, 2, 0, 1).reshape(128, NC, CS)
            ).astype(NPBF16),
            # [t, ii, d] -> [ii, t, d]
            "cx8": np.ascontiguousarray(
                cb.reshape(NT, 128, D).transpose(1, 0, 2)
            ).astype(NPBF16),
            "q": qb.astype(NPBF16),
            # qT [d, j]: [c, p, j] -> [p, c, j]
            "qT8": np.ascontiguousarray(
                qb.T.reshape(NC, 128, QS).transpose(1, 0, 2)
            ).astype(NPBF16),
        }
        m.update(shared)
        in_maps.append(m)

    nc = _get_nc()
    res = run_bass_kernel_spmd(nc, in_maps, list(range(B)), trace=TRACE)
    _LAST_EXEC_NS = res.exec_time_ns
    out = np.stack(
        [res.results[b]["out"].astype(np.float32) for b in range(B)], axis=0
    )
    return out
